# revision 24
# baseline (speedup 1.0000x reference)
"""DIFSR attention kernel for Trainium2, 8 NeuronCores, data-parallel over batch.

Math (per batch b):
  S_h = (Xid Wq_id)(Xid Wk_id)^T*s + (Xc Wq_c)(Xc Wk_c)^T*s + (Xp Wq_p)(Xp Wk_p)^T*s
        + rel_time_h + mask_add                       (s = HD^-0.5, folded into Q scale/bias)
  A_h = softmax_k(S_h);  O_h = A_h V_h;  y = concat_h(O_h) Wo + bo

Device dataflow is fully "transposed-activation" so no on-chip transposes exist:
  - host pre-transposes inputs to xT [HID, L], rel_time to [k, q] layout (mask
    folded in as -30000), and pre-swizzles every tensor into the exact SBUF
    partition-major layout so all DMAs are linear (>=4KB contiguous per partition),
  - projections produce QT/KT [d, q] directly (weights stationary),
  - scores are computed as S^T [k, q] (K stationary), two heads packed into the
    128-partition dim via tile_position row groups (contraction K=64 each),
  - softmax denominator comes free from the PV matmul via a ones column
    appended to V ([65, q] PSUM row 64 = sum_k E^T[k, q]),
  - exp uses a fixed shift (no row max): attn = E/D is shift-invariant,
  - PV consumes E^T directly producing O^T; out-proj consumes O^T producing y
    in natural layout for a contiguous store.

The emission order software-pipelines the PE queue: each head-pair's last PV
matmul and normalize are deferred until after the next pair's projection
matmuls so the PE never waits on the DVE/ACT softmax chain.

Precision: fp16 operands with fp32 PSUM accumulation; score+rel add, exp and
1/D in fp32.  Measured absmax-relative error vs the fp32 reference ~5.5e-4.
"""

import numpy as np

B, L, HID, NH, HD = 16, 512, 1024, 16, 64
NCORES = 8
BPC = B // NCORES  # batches per core
SHIFT = 4.0        # exp(s - SHIFT): keeps E in fp16 range for this data regime
MASKVAL = -30000.0
KT = HID // 128    # 8 contraction tiles
NJ = NH // 2       # 8 head pairs

_CACHE = {}


def build_bass():
    import concourse.bass as bass
    import concourse.mybir as mybir
    import concourse.tile as tile
    from concourse import bacc
    from contextlib import ExitStack

    f16 = mybir.dt.float16
    f32 = mybir.dt.float32
    AF = mybir.ActivationFunctionType

    nc = bacc.Bacc()

    # All inputs are host-preswizzled to partition-major layouts (dim holding
    # 128 comes first; the rest is contiguous per partition) for linear DMA.
    xt = nc.dram_tensor("xt", [4, BPC, 128, KT, L], f16, kind="ExternalInput")
    wqk = nc.dram_tensor("wqk", [NJ, 128, 6, KT, 128], f16, kind="ExternalInput")
    wv = nc.dram_tensor("wv", [128, KT, HID], f16, kind="ExternalInput")
    wo = nc.dram_tensor("wo", [128, KT, HID], f16, kind="ExternalInput")
    bqk = nc.dram_tensor("bqk", [128, 6, KT], f16, kind="ExternalInput")
    bv = nc.dram_tensor("bv", [HID], f16, kind="ExternalInput")
    bo = nc.dram_tensor("bo", [HID], f16, kind="ExternalInput")
    relt = nc.dram_tensor("relt", [BPC, NH, 128, 4, L], f16, kind="ExternalInput")
    y = nc.dram_tensor("y", [BPC, L, HID], f32, kind="ExternalOutput")

    with tile.TileContext(nc) as tc, ExitStack() as ctx:
        persist = ctx.enter_context(tc.tile_pool(name="persist", bufs=1))
        wslices = ctx.enter_context(tc.tile_pool(name="wslices", bufs=2))
        qkt_p = ctx.enter_context(tc.tile_pool(name="qkt", bufs=12))
        rel_p = ctx.enter_context(tc.tile_pool(name="relp", bufs=3))
        e_p = ctx.enter_context(tc.tile_pool(name="ep", bufs=5))
        rc_p = ctx.enter_context(tc.tile_pool(name="rcp", bufs=2))
        bc_p = ctx.enter_context(tc.tile_pool(name="bcp", bufs=2))
        ysb_p = ctx.enter_context(tc.tile_pool(name="ysb", bufs=2))
        ps_big = ctx.enter_context(tc.tile_pool(name="psbig", bufs=2, space="PSUM"))
        ps_s = ctx.enter_context(tc.tile_pool(name="pss", bufs=4, space="PSUM"))
        ps_o = ctx.enter_context(tc.tile_pool(name="pso", bufs=2, space="PSUM"))

        # ---- resident tiles ----
        xt_all = persist.tile([128, 3, BPC, KT, L], f16, tag="xt_all")
        wv_sb = persist.tile([128, KT, HID], f16, tag="wv_sb")
        wo_sb = persist.tile([128, KT, HID], f16, tag="wo_sb")
        bqk_sb = persist.tile([128, 6, KT], f16, tag="bqk_sb")
        bv_sb = persist.tile([1, HID], f16, tag="bv_sb")
        bo_sb = persist.tile([1, HID], f16, tag="bo_sb")
        ones1 = persist.tile([1, 128], f16, tag="ones1")
        expb = persist.tile([128, 1], f32, tag="expb")
        v_aug = persist.tile([128, BPC, 4, 16 * 80], f16, tag="v_aug")
        ot_all = persist.tile([128, BPC, NJ, L], f16, tag="ot_all")

        nc.vector.memset(ones1[:], 1.0)
        nc.vector.memset(expb[:], -SHIFT)

        # ---- V projection: V[q, n] (natural layout), packed as [q, 16*(64+1)]
        # with a ones column per head for the softmax denominator.  The V input
        # tile lives in its own pool, released after this phase. ----
        with tc.tile_pool(name="xtv", bufs=1) as xtv_pool:
            xt_v = xtv_pool.tile([128, BPC, KT, L], f16, tag="xt_v")
            for b in range(BPC):
                nc.sync.dma_start(out=xt_v[:, b], in_=xt[3, b])
            nc.sync.dma_start(out=wv_sb[:], in_=wv[:])
            for src in range(3):
                for b in range(BPC):
                    nc.sync.dma_start(out=xt_all[:, src, b], in_=xt[src, b])
            nc.sync.dma_start(out=wo_sb[:], in_=wo[:])
            nc.sync.dma_start(out=bqk_sb[:], in_=bqk[:])
            nc.sync.dma_start(out=bv_sb[:], in_=bv[None, :])
            nc.sync.dma_start(out=bo_sb[:], in_=bo[None, :])

            for b in range(BPC):
                v_aug_b = v_aug[:, b].rearrange("p t (h c) -> p t h c", c=80)
                for qt in range(4):
                    nc.vector.memset(v_aug_b[:, qt, :, 64:65], 1.0)
                    for nh in range(2):
                        ps = ps_big.tile([128, 512], f32, tag="psbig", name="psv")
                        for kt in range(KT):
                            nc.tensor.matmul(
                                ps[:],
                                lhsT=xt_v[:, b, kt, qt * 128:(qt + 1) * 128],
                                rhs=wv_sb[:, kt, nh * 512:(nh + 1) * 512],
                                start=(kt == 0), stop=False,
                            )
                        nc.tensor.matmul(
                            ps[:], lhsT=ones1[:], rhs=bv_sb[:, nh * 512:(nh + 1) * 512],
                            start=False, stop=True,
                        )
                        nc.vector.tensor_copy(
                            v_aug_b[:, qt, nh * 8:(nh + 1) * 8, 0:64],
                            ps[:].rearrange("p (h d) -> p h d", d=64),
                        )

        # ---- per head-pair pipeline ----
        def emit_proj(wsl, j, b):
            """Six projections (Qid,Kid,Qc,Kc,Qp,Kp) for head pair j, batch b."""
            qk = []
            for w6 in range(6):
                src = w6 // 2
                ps = ps_big.tile([128, 512], f32, tag="psbig", name="psp")
                for kt in range(KT):
                    nc.tensor.matmul(
                        ps[:],
                        lhsT=wsl[:, w6, kt],
                        rhs=xt_all[:, src, b, kt],
                        start=(kt == 0), stop=(kt == KT - 1),
                    )
                t = qkt_p.tile([128, 512], f16, tag="qkt", name="qkt")
                is_q = (w6 % 2 == 0)
                nc.scalar.activation(
                    t[:], ps[:], AF.Identity,
                    bias=bqk_sb[:, w6, j:j + 1],
                    scale=(float(HD) ** -0.5 if is_q else 1.0),
                )
                qk.append(t)
            return qk

        def emit_scores(qk, pss, kts):
            for si in range(3):
                for h01 in range(2):
                    sl = slice(64 * h01, 64 * (h01 + 1))
                    nc.tensor.matmul(
                        pss[h01][:],
                        lhsT=qk[2 * si + 1][sl, kts * 128:(kts + 1) * 128],
                        rhs=qk[2 * si][sl, :],
                        start=(si == 0), stop=(si == 2),
                        tile_position=(64 * h01, 0),
                    )

        def emit_softmax(pss, rel, kts):
            es = []
            for h01 in range(2):
                nc.vector.tensor_add(pss[h01][:], pss[h01][:], rel[h01][:, kts])
                e = e_p.tile([128, 512], f16, tag="ep", name="e")
                nc.scalar.activation(e[:], pss[h01][:], AF.Exp, bias=expb[:])
                es.append(e)
            return es

        def emit_pv(po, es, v_aug_b, j, b, kts):
            for h01 in range(2):
                nc.tensor.matmul(
                    po[h01][:],
                    lhsT=v_aug_b[:, kts, 2 * j + h01, 0:65],
                    rhs=es[h01][:],
                    start=(kts == 0), stop=(kts == 3),
                )

        def emit_normalize(po, j, b):
            for h01 in range(2):
                # 1/D = exp(-ln(D)) on the ACT engine: keeps the slow iterative
                # reciprocal off the DVE, whose FIFO also carries the softmax adds.
                ln = rc_p.tile([1, 512], f32, tag="lnp", name="ln")
                nc.scalar.activation(ln[:], po[h01][64:65, :], AF.Ln)
                rc = rc_p.tile([1, 512], f32, tag="rcp", name="rc")
                nc.scalar.activation(rc[:], ln[:], AF.Exp, scale=-1.0)
                bc = bc_p.tile([64, 512], f32, tag="bcp", name="bc")
                nc.sync.dma_start(
                    out=bc[:], in_=rc[0:1, None, :].broadcast_to([1, 64, 512])
                )
                nc.vector.tensor_mul(
                    ot_all[64 * h01:64 * (h01 + 1), b, j, :],
                    po[h01][0:64, :],
                    bc[:],
                )

        def finish_pair(pending):
            """Deferred last PV matmul + normalize for the previous head pair —
            emitted after the next pair's projection matmuls so the PE queue
            never head-of-line blocks on the softmax chain."""
            ppo, pes, pj, pb = pending
            emit_pv(
                ppo, pes,
                v_aug[:, pb].rearrange("p t (h c) -> p t h c", c=80),
                pj, pb, 3,
            )
            emit_normalize(ppo, pj, pb)

        wsl = None
        pending = None
        for j in range(NJ):
            for b in range(BPC):
                if b == 0:
                    wsl = wslices.tile([128, 6, KT, 128], f16, tag="wsl", name="wsl")
                    nc.sync.dma_start(out=wsl[:], in_=wqk[j])
                qk = emit_proj(wsl, j, b)

                if pending is not None:
                    finish_pair(pending)

                rel = []
                for h01 in range(2):
                    rt = rel_p.tile([128, 4, 512], f16, tag="relp", name="rel")
                    nc.sync.dma_start(out=rt[:], in_=relt[b, 2 * j + h01])
                    rel.append(rt)

                v_aug_b = v_aug[:, b].rearrange("p t (h c) -> p t h c", c=80)
                po = [ps_o.tile([65, 512], f32, tag="pso", name="po") for _ in range(2)]
                es_by_kts = []
                for kts in range(4):
                    pss = [ps_s.tile([128, 512], f32, tag="pss", name="pss") for _ in range(2)]
                    emit_scores(qk, pss, kts)
                    es_by_kts.append(emit_softmax(pss, rel, kts))
                    if kts >= 1:
                        emit_pv(po, es_by_kts[kts - 1], v_aug_b, j, b, kts - 1)
                pending = (po, es_by_kts[3], j, b)
        finish_pair(pending)

        # ---- output projection: y[q, n] ----
        for b in range(BPC):
            for qt in range(4):
                for nh in range(2):
                    ps = ps_big.tile([128, 512], f32, tag="psbig", name="psy")
                    for jj in range(NJ):
                        nc.tensor.matmul(
                            ps[:],
                            lhsT=ot_all[:, b, jj, qt * 128:(qt + 1) * 128],
                            rhs=wo_sb[:, jj, nh * 512:(nh + 1) * 512],
                            start=(jj == 0), stop=False,
                        )
                    nc.tensor.matmul(
                        ps[:], lhsT=ones1[:], rhs=bo_sb[:, nh * 512:(nh + 1) * 512],
                        start=False, stop=True,
                    )
                    ysb = ysb_p.tile([128, 512], f32, tag="ysb", name="ysb")
                    nc.scalar.copy(ysb[:], ps[:])
                    nc.sync.dma_start(
                        out=y[b, qt * 128:(qt + 1) * 128, nh * 512:(nh + 1) * 512],
                        in_=ysb[:],
                    )

    nc.finalize()
    return nc


def prep_inputs(inputs):
    """Host-side sharding + layout prep. Returns per-core in_maps.

    Every device tensor is laid out partition-major so DMAs are linear:
    the value at SBUF (partition p, ...) sits contiguously in DRAM.
    """
    f16 = np.float16
    inputs = {k: np.asarray(v) for k, v in inputs.items()}
    s = float(HD) ** -0.5

    # xt: [4, B, 128p, KT, L] where (kt*128+p) indexes HID of x^T [HID, L]
    xt_full = np.empty((4, B, 128, KT, L), f16)
    for i, k in enumerate(("seq_id", "seq_cate", "seq_pos", "V_id_input")):
        x = inputs[k].astype(f16)                       # [B, L, HID]
        xt = x.transpose(0, 2, 1)                       # [B, HID, L]
        xt_full[i] = xt.reshape(B, KT, 128, L).transpose(0, 2, 1, 3)

    # wqk: [NJ, 128p, 6, KT, 128n] — per head-pair column slices of the six
    # Q/K weight matrices, hid_in = kt*128+p.
    wqk_st = np.stack(
        [inputs[k] for k in ("q_id_w", "k_id_w", "q_cate_w", "k_cate_w", "q_pos_w", "k_pos_w")]
    ).astype(f16)                                       # [6, HID, HID]
    wqk_r = wqk_st.reshape(6, KT, 128, NJ, 128)          # [6, kt, p, j, n]
    wqk_lin = np.ascontiguousarray(wqk_r.transpose(3, 2, 0, 1, 4))  # [j, p, 6, kt, n]

    def w_lin(w):  # [HID, HID] -> [128p, KT, HID]
        return np.ascontiguousarray(
            w.astype(f16).reshape(KT, 128, HID).transpose(1, 0, 2)
        )

    wv_lin = w_lin(inputs["v_id_w"])
    wo_lin = w_lin(inputs["out_w"])

    bqk_st = np.stack(
        [
            inputs["q_id_b"] * s, inputs["k_id_b"],
            inputs["q_cate_b"] * s, inputs["k_cate_b"],
            inputs["q_pos_b"] * s, inputs["k_pos_b"],
        ]
    ).astype(f16)                                       # [6, HID]
    bqk_lin = np.ascontiguousarray(
        bqk_st.reshape(6, KT, 128).transpose(2, 0, 1)   # [128p, 6, kt]
    )
    bv_h = inputs["v_id_b"].astype(f16)
    bo_h = inputs["out_b"].astype(f16)

    # relt: [B, NH, 128p, 4kts, L] with (kts*128+p) indexing k of rel^T [k, q]
    relT = np.empty((B, NH, 128, 4, L), f16)
    for b in range(B):
        maskadd = np.where(inputs["attn_mask"][b], np.float32(0), np.float32(MASKVAL))
        relb = inputs["relative_time"][b].astype(np.float32) + maskadd[None]
        rT = relb.transpose(0, 2, 1).astype(f16)         # [NH, k, q]
        relT[b] = rT.reshape(NH, 4, 128, L).transpose(0, 2, 1, 3)

    in_maps = []
    for c in range(NCORES):
        bs = slice(c * BPC, (c + 1) * BPC)
        in_maps.append(
            {
                "xt": np.ascontiguousarray(xt_full[:, bs]),
                "wqk": wqk_lin, "wv": wv_lin, "wo": wo_lin,
                "bqk": bqk_lin, "bv": bv_h, "bo": bo_h,
                "relt": np.ascontiguousarray(relT[bs]),
            }
        )
    return in_maps


def kernel(**inputs):
    from concourse.bass_utils import run_bass_kernel_spmd

    if "nc" not in _CACHE:
        _CACHE["nc"] = build_bass()
    nc = _CACHE["nc"]
    in_maps = prep_inputs(inputs)
    res = run_bass_kernel_spmd(nc, in_maps, list(range(NCORES)))
    out = np.concatenate([res.results[c]["y"] for c in range(NCORES)], axis=0)
    return out.astype(np.float32)


# revision 25
# speedup vs baseline: 1.0632x; 1.0632x over previous
"""DIFSR attention kernel for Trainium2, 8 NeuronCores, data-parallel over batch.

Math (per batch b):
  S_h = (Xid Wq_id)(Xid Wk_id)^T*s + (Xc Wq_c)(Xc Wk_c)^T*s + (Xp Wq_p)(Xp Wk_p)^T*s
        + rel_time_h + mask_add                       (s = HD^-0.5, folded into Q scale/bias)
  A_h = softmax_k(S_h);  O_h = A_h V_h;  y = concat_h(O_h) Wo + bo

Device dataflow is fully "transposed-activation" so no on-chip transposes exist:
  - host pre-transposes inputs to xT [HID, L], rel_time to [k, q] layout (mask
    folded in as -30000), and pre-swizzles every tensor into the exact SBUF
    partition-major layout so all DMAs are linear (>=4KB contiguous per partition),
  - projections produce QT/KT [d, q] directly (weights stationary),
  - scores are computed as S^T [k, q] (K stationary), two heads packed into the
    128-partition dim via tile_position row groups (contraction K=64 each),
  - softmax denominator comes free from the PV matmul via a ones column
    appended to V ([65, q] PSUM row 64 = sum_k E^T[k, q]),
  - exp uses a fixed shift (no row max): attn = E/D is shift-invariant,
  - PV consumes E^T directly producing O^T; out-proj consumes O^T producing y
    in natural layout for a contiguous store.

The emission order software-pipelines the PE queue: each head-pair's last PV
matmul and normalize are deferred until after the next pair's projection
matmuls so the PE never waits on the DVE/ACT softmax chain.

Precision: fp16 operands with fp32 PSUM accumulation; score+rel add, exp and
1/D in fp32.  Measured absmax-relative error vs the fp32 reference ~5.5e-4.
"""

import numpy as np

B, L, HID, NH, HD = 16, 512, 1024, 16, 64
NCORES = 8
BPC = B // NCORES  # batches per core
SHIFT = 4.0        # exp(s - SHIFT): keeps E in fp16 range for this data regime
MASKVAL = -30000.0
KT = HID // 128    # 8 contraction tiles
NJ = NH // 2       # 8 head pairs

_CACHE = {}


def build_bass():
    import concourse.bass as bass
    import concourse.mybir as mybir
    import concourse.tile as tile
    from concourse import bacc
    from contextlib import ExitStack

    f16 = mybir.dt.float16
    f32 = mybir.dt.float32
    AF = mybir.ActivationFunctionType

    nc = bacc.Bacc()

    # All inputs are host-preswizzled to partition-major layouts (dim holding
    # 128 comes first; the rest is contiguous per partition) for linear DMA.
    xt = nc.dram_tensor("xt", [4, BPC, 128, KT, L], f16, kind="ExternalInput")
    wqk = nc.dram_tensor("wqk", [NJ, 128, 6, KT, 128], f16, kind="ExternalInput")
    wv = nc.dram_tensor("wv", [128, KT, HID], f16, kind="ExternalInput")
    wo = nc.dram_tensor("wo", [128, KT, HID], f16, kind="ExternalInput")
    bqk = nc.dram_tensor("bqk", [128, 6, KT], f16, kind="ExternalInput")
    bv = nc.dram_tensor("bv", [HID], f16, kind="ExternalInput")
    bo = nc.dram_tensor("bo", [HID], f16, kind="ExternalInput")
    relt = nc.dram_tensor("relt", [BPC, NH, 128, 4, L], f16, kind="ExternalInput")
    y = nc.dram_tensor("y", [BPC, L, HID], f32, kind="ExternalOutput")

    with tile.TileContext(nc) as tc, ExitStack() as ctx:
        persist = ctx.enter_context(tc.tile_pool(name="persist", bufs=1))
        wslices = ctx.enter_context(tc.tile_pool(name="wslices", bufs=2))
        qkt_p = ctx.enter_context(tc.tile_pool(name="qkt", bufs=12))
        rel_p = ctx.enter_context(tc.tile_pool(name="relp", bufs=3))
        e_p = ctx.enter_context(tc.tile_pool(name="ep", bufs=5))
        rc_p = ctx.enter_context(tc.tile_pool(name="rcp", bufs=2))
        bc_p = ctx.enter_context(tc.tile_pool(name="bcp", bufs=2))
        ysb_p = ctx.enter_context(tc.tile_pool(name="ysb", bufs=2))
        ps_big = ctx.enter_context(tc.tile_pool(name="psbig", bufs=2, space="PSUM"))
        ps_s = ctx.enter_context(tc.tile_pool(name="pss", bufs=4, space="PSUM"))
        ps_o = ctx.enter_context(tc.tile_pool(name="pso", bufs=2, space="PSUM"))

        # ---- resident tiles ----
        xt_all = persist.tile([128, 3, BPC, KT, L], f16, tag="xt_all")
        wv_sb = persist.tile([128, KT, HID], f16, tag="wv_sb")
        wo_sb = persist.tile([128, KT, HID], f16, tag="wo_sb")
        bqk_sb = persist.tile([128, 6, KT], f16, tag="bqk_sb")
        bv_sb = persist.tile([1, HID], f16, tag="bv_sb")
        bo_sb = persist.tile([1, HID], f16, tag="bo_sb")
        ones1 = persist.tile([1, 128], f16, tag="ones1")
        expb = persist.tile([128, 1], f32, tag="expb")
        v_aug = persist.tile([128, BPC, 4, 16 * 80], f16, tag="v_aug")
        ot_all = persist.tile([128, BPC, NJ, L], f16, tag="ot_all")

        nc.vector.memset(ones1[:], 1.0)
        nc.vector.memset(expb[:], -SHIFT)

        # ---- V projection: V[q, n] (natural layout), packed as [q, 16*(64+1)]
        # with a ones column per head for the softmax denominator.  The V input
        # tile lives in its own pool, released after this phase. ----
        with tc.tile_pool(name="xtv", bufs=1) as xtv_pool:
            xt_v = xtv_pool.tile([128, BPC, KT, L], f16, tag="xt_v")
            for b in range(BPC):
                nc.sync.dma_start(out=xt_v[:, b], in_=xt[3, b])
            nc.sync.dma_start(out=wv_sb[:], in_=wv[:])
            for src in range(3):
                for b in range(BPC):
                    nc.sync.dma_start(out=xt_all[:, src, b], in_=xt[src, b])
            nc.sync.dma_start(out=wo_sb[:], in_=wo[:])
            nc.sync.dma_start(out=bqk_sb[:], in_=bqk[:])
            nc.sync.dma_start(out=bv_sb[:], in_=bv[None, :])
            nc.sync.dma_start(out=bo_sb[:], in_=bo[None, :])

            for b in range(BPC):
                v_aug_b = v_aug[:, b].rearrange("p t (h c) -> p t h c", c=80)
                for qt in range(4):
                    nc.vector.memset(v_aug_b[:, qt, :, 64:65], 1.0)
                    for nh in range(2):
                        ps = ps_big.tile([128, 512], f32, tag="psbig", name="psv")
                        for kt in range(KT):
                            nc.tensor.matmul(
                                ps[:],
                                lhsT=xt_v[:, b, kt, qt * 128:(qt + 1) * 128],
                                rhs=wv_sb[:, kt, nh * 512:(nh + 1) * 512],
                                start=(kt == 0), stop=False,
                            )
                        nc.tensor.matmul(
                            ps[:], lhsT=ones1[:], rhs=bv_sb[:, nh * 512:(nh + 1) * 512],
                            start=False, stop=True,
                        )
                        nc.vector.tensor_copy(
                            v_aug_b[:, qt, nh * 8:(nh + 1) * 8, 0:64],
                            ps[:].rearrange("p (h d) -> p h d", d=64),
                        )

        # ---- per head-pair pipeline ----
        def emit_proj(wsl, j, b):
            """Six projections (Qid,Kid,Qc,Kc,Qp,Kp) for head pair j, batch b."""
            qk = []
            for w6 in range(6):
                src = w6 // 2
                ps = ps_big.tile([128, 512], f32, tag="psbig", name="psp")
                for kt in range(KT):
                    nc.tensor.matmul(
                        ps[:],
                        lhsT=wsl[:, w6, kt],
                        rhs=xt_all[:, src, b, kt],
                        start=(kt == 0), stop=(kt == KT - 1),
                    )
                t = qkt_p.tile([128, 512], f16, tag="qkt", name="qkt")
                is_q = (w6 % 2 == 0)
                nc.scalar.activation(
                    t[:], ps[:], AF.Identity,
                    bias=bqk_sb[:, w6, j:j + 1],
                    scale=(float(HD) ** -0.5 if is_q else 1.0),
                )
                qk.append(t)
            return qk

        def emit_scores(qk, pss, kts):
            for si in range(3):
                for h01 in range(2):
                    sl = slice(64 * h01, 64 * (h01 + 1))
                    nc.tensor.matmul(
                        pss[h01][:],
                        lhsT=qk[2 * si + 1][sl, kts * 128:(kts + 1) * 128],
                        rhs=qk[2 * si][sl, :],
                        start=(si == 0), stop=(si == 2),
                        tile_position=(64 * h01, 0),
                    )

        def emit_softmax(pss, rel, kts):
            es = []
            for h01 in range(2):
                nc.vector.tensor_add(pss[h01][:], pss[h01][:], rel[h01][:, kts])
                e = e_p.tile([128, 512], f16, tag="ep", name="e")
                nc.scalar.activation(e[:], pss[h01][:], AF.Exp, bias=expb[:])
                es.append(e)
            return es

        def emit_pv(po, es, v_aug_b, j, b, kts):
            for h01 in range(2):
                nc.tensor.matmul(
                    po[h01][:],
                    lhsT=v_aug_b[:, kts, 2 * j + h01, 0:65],
                    rhs=es[h01][:],
                    start=(kts == 0), stop=(kts == 3),
                )

        def emit_normalize(po, j, b):
            for h01 in range(2):
                # Stage D into SBUF on ACT (plain copy, no table switch), then the
                # fast seed+Newton reciprocal on DVE (custom op needs SBUF input).
                dsb = rc_p.tile([1, 512], f32, tag="dsb", name="dsb")
                nc.scalar.copy(dsb[:], po[h01][64:65, :])
                rc = rc_p.tile([1, 512], f32, tag="rcp", name="rc")
                nc.vector.reciprocal_approx_fast(rc[:], dsb[:])
                bc = bc_p.tile([64, 512], f32, tag="bcp", name="bc")
                nc.sync.dma_start(
                    out=bc[:], in_=rc[0:1, None, :].broadcast_to([1, 64, 512])
                )
                nc.vector.tensor_mul(
                    ot_all[64 * h01:64 * (h01 + 1), b, j, :],
                    po[h01][0:64, :],
                    bc[:],
                )

        def finish_pair(pending):
            """Deferred last PV matmul + normalize for the previous head pair —
            emitted after the next pair's projection matmuls so the PE queue
            never head-of-line blocks on the softmax chain."""
            ppo, pes, pj, pb = pending
            emit_pv(
                ppo, pes,
                v_aug[:, pb].rearrange("p t (h c) -> p t h c", c=80),
                pj, pb, 3,
            )
            emit_normalize(ppo, pj, pb)

        wsl = None
        pending = None
        for j in range(NJ):
            for b in range(BPC):
                if b == 0:
                    wsl = wslices.tile([128, 6, KT, 128], f16, tag="wsl", name="wsl")
                    nc.sync.dma_start(out=wsl[:], in_=wqk[j])
                qk = emit_proj(wsl, j, b)

                if pending is not None:
                    finish_pair(pending)

                rel = []
                for h01 in range(2):
                    rt = rel_p.tile([128, 4, 512], f16, tag="relp", name="rel")
                    nc.sync.dma_start(out=rt[:], in_=relt[b, 2 * j + h01])
                    rel.append(rt)

                v_aug_b = v_aug[:, b].rearrange("p t (h c) -> p t h c", c=80)
                po = [ps_o.tile([65, 512], f32, tag="pso", name="po") for _ in range(2)]
                es_by_kts = []
                for kts in range(4):
                    pss = [ps_s.tile([128, 512], f32, tag="pss", name="pss") for _ in range(2)]
                    emit_scores(qk, pss, kts)
                    es_by_kts.append(emit_softmax(pss, rel, kts))
                    if kts >= 1:
                        emit_pv(po, es_by_kts[kts - 1], v_aug_b, j, b, kts - 1)
                pending = (po, es_by_kts[3], j, b)
        finish_pair(pending)

        # ---- output projection: y[q, n] ----
        for b in range(BPC):
            for qt in range(4):
                for nh in range(2):
                    ps = ps_big.tile([128, 512], f32, tag="psbig", name="psy")
                    for jj in range(NJ):
                        nc.tensor.matmul(
                            ps[:],
                            lhsT=ot_all[:, b, jj, qt * 128:(qt + 1) * 128],
                            rhs=wo_sb[:, jj, nh * 512:(nh + 1) * 512],
                            start=(jj == 0), stop=False,
                        )
                    nc.tensor.matmul(
                        ps[:], lhsT=ones1[:], rhs=bo_sb[:, nh * 512:(nh + 1) * 512],
                        start=False, stop=True,
                    )
                    ysb = ysb_p.tile([128, 512], f32, tag="ysb", name="ysb")
                    nc.scalar.copy(ysb[:], ps[:])
                    nc.sync.dma_start(
                        out=y[b, qt * 128:(qt + 1) * 128, nh * 512:(nh + 1) * 512],
                        in_=ysb[:],
                    )

    nc.finalize()
    return nc


def prep_inputs(inputs):
    """Host-side sharding + layout prep. Returns per-core in_maps.

    Every device tensor is laid out partition-major so DMAs are linear:
    the value at SBUF (partition p, ...) sits contiguously in DRAM.
    """
    f16 = np.float16
    inputs = {k: np.asarray(v) for k, v in inputs.items()}
    s = float(HD) ** -0.5

    # xt: [4, B, 128p, KT, L] where (kt*128+p) indexes HID of x^T [HID, L]
    xt_full = np.empty((4, B, 128, KT, L), f16)
    for i, k in enumerate(("seq_id", "seq_cate", "seq_pos", "V_id_input")):
        x = inputs[k].astype(f16)                       # [B, L, HID]
        xt = x.transpose(0, 2, 1)                       # [B, HID, L]
        xt_full[i] = xt.reshape(B, KT, 128, L).transpose(0, 2, 1, 3)

    # wqk: [NJ, 128p, 6, KT, 128n] — per head-pair column slices of the six
    # Q/K weight matrices, hid_in = kt*128+p.
    wqk_st = np.stack(
        [inputs[k] for k in ("q_id_w", "k_id_w", "q_cate_w", "k_cate_w", "q_pos_w", "k_pos_w")]
    ).astype(f16)                                       # [6, HID, HID]
    wqk_r = wqk_st.reshape(6, KT, 128, NJ, 128)          # [6, kt, p, j, n]
    wqk_lin = np.ascontiguousarray(wqk_r.transpose(3, 2, 0, 1, 4))  # [j, p, 6, kt, n]

    def w_lin(w):  # [HID, HID] -> [128p, KT, HID]
        return np.ascontiguousarray(
            w.astype(f16).reshape(KT, 128, HID).transpose(1, 0, 2)
        )

    wv_lin = w_lin(inputs["v_id_w"])
    wo_lin = w_lin(inputs["out_w"])

    bqk_st = np.stack(
        [
            inputs["q_id_b"] * s, inputs["k_id_b"],
            inputs["q_cate_b"] * s, inputs["k_cate_b"],
            inputs["q_pos_b"] * s, inputs["k_pos_b"],
        ]
    ).astype(f16)                                       # [6, HID]
    bqk_lin = np.ascontiguousarray(
        bqk_st.reshape(6, KT, 128).transpose(2, 0, 1)   # [128p, 6, kt]
    )
    bv_h = inputs["v_id_b"].astype(f16)
    bo_h = inputs["out_b"].astype(f16)

    # relt: [B, NH, 128p, 4kts, L] with (kts*128+p) indexing k of rel^T [k, q]
    relT = np.empty((B, NH, 128, 4, L), f16)
    for b in range(B):
        maskadd = np.where(inputs["attn_mask"][b], np.float32(0), np.float32(MASKVAL))
        relb = inputs["relative_time"][b].astype(np.float32) + maskadd[None]
        rT = relb.transpose(0, 2, 1).astype(f16)         # [NH, k, q]
        relT[b] = rT.reshape(NH, 4, 128, L).transpose(0, 2, 1, 3)

    in_maps = []
    for c in range(NCORES):
        bs = slice(c * BPC, (c + 1) * BPC)
        in_maps.append(
            {
                "xt": np.ascontiguousarray(xt_full[:, bs]),
                "wqk": wqk_lin, "wv": wv_lin, "wo": wo_lin,
                "bqk": bqk_lin, "bv": bv_h, "bo": bo_h,
                "relt": np.ascontiguousarray(relT[bs]),
            }
        )
    return in_maps


def kernel(**inputs):
    from concourse.bass_utils import run_bass_kernel_spmd

    if "nc" not in _CACHE:
        _CACHE["nc"] = build_bass()
    nc = _CACHE["nc"]
    in_maps = prep_inputs(inputs)
    res = run_bass_kernel_spmd(nc, in_maps, list(range(NCORES)))
    out = np.concatenate([res.results[c]["y"] for c in range(NCORES)], axis=0)
    return out.astype(np.float32)


# revision 27
# speedup vs baseline: 1.1215x; 1.0548x over previous
"""DIFSR attention kernel for Trainium2, 8 NeuronCores, data-parallel over batch.

Math (per batch b):
  S_h = (Xid Wq_id)(Xid Wk_id)^T*s + (Xc Wq_c)(Xc Wk_c)^T*s + (Xp Wq_p)(Xp Wk_p)^T*s
        + rel_time_h + mask_add                       (s = HD^-0.5, folded into Q scale/bias)
  A_h = softmax_k(S_h);  O_h = A_h V_h;  y = concat_h(O_h) Wo + bo

Device dataflow is fully "transposed-activation" so no on-chip transposes exist:
  - host pre-transposes inputs to xT [HID, L], rel_time to [k, q] layout (mask
    folded in as -30000), and pre-swizzles every tensor into the exact SBUF
    partition-major layout so all DMAs are linear (>=4KB contiguous per partition),
  - projections produce QT/KT [d, q] directly (weights stationary),
  - scores are computed as S^T [k, q] (K stationary), two heads packed into the
    128-partition dim via tile_position row groups (contraction K=64 each),
  - softmax denominator comes free from the PV matmul via a ones column
    appended to V ([65, q] PSUM row 64 = sum_k E^T[k, q]),
  - exp uses a fixed shift (no row max): attn = E/D is shift-invariant,
  - PV consumes E^T directly producing O^T; out-proj consumes O^T producing y
    in natural layout for a contiguous store.

The emission order software-pipelines the PE queue: each head-pair's last PV
matmul and normalize are deferred until after the next pair's projection
matmuls so the PE never waits on the DVE/ACT softmax chain.

Precision: fp16 operands with fp32 PSUM accumulation; score+rel add, exp and
1/D in fp32.  Measured absmax-relative error vs the fp32 reference ~5.5e-4.
"""

import numpy as np

B, L, HID, NH, HD = 16, 512, 1024, 16, 64
NCORES = 8
BPC = B // NCORES  # batches per core
SHIFT = 4.0        # exp(s - SHIFT): keeps E in fp16 range for this data regime
MASKVAL = -30000.0
KT = HID // 128    # 8 contraction tiles
NJ = NH // 2       # 8 head pairs

_CACHE = {}


def build_bass():
    import concourse.bass as bass
    import concourse.mybir as mybir
    import concourse.tile as tile
    from concourse import bacc
    from contextlib import ExitStack

    f16 = mybir.dt.float16
    f32 = mybir.dt.float32
    AF = mybir.ActivationFunctionType

    nc = bacc.Bacc()

    # All inputs are host-preswizzled to partition-major layouts (dim holding
    # 128 comes first; the rest is contiguous per partition) for linear DMA.
    xt = nc.dram_tensor("xt", [4, BPC, 128, KT, L], f16, kind="ExternalInput")
    wqk = nc.dram_tensor("wqk", [NJ, 128, 6, KT, 128], f16, kind="ExternalInput")
    wv = nc.dram_tensor("wv", [128, KT, HID], f16, kind="ExternalInput")
    wo = nc.dram_tensor("wo", [128, KT, HID], f16, kind="ExternalInput")
    bqk = nc.dram_tensor("bqk", [128, 6, KT], f16, kind="ExternalInput")
    bv = nc.dram_tensor("bv", [HID], f16, kind="ExternalInput")
    bo = nc.dram_tensor("bo", [HID], f16, kind="ExternalInput")
    relt = nc.dram_tensor("relt", [BPC, NH, 128, 4, L], f16, kind="ExternalInput")
    y = nc.dram_tensor("y", [BPC, L, HID], f32, kind="ExternalOutput")

    with tile.TileContext(nc) as tc, ExitStack() as ctx:
        persist = ctx.enter_context(tc.tile_pool(name="persist", bufs=1))
        wslices = ctx.enter_context(tc.tile_pool(name="wslices", bufs=2))
        qkt_p = ctx.enter_context(tc.tile_pool(name="qkt", bufs=11))
        rel_p = ctx.enter_context(tc.tile_pool(name="relp", bufs=3))
        e_p = ctx.enter_context(tc.tile_pool(name="ep", bufs=4))
        rc_p = ctx.enter_context(tc.tile_pool(name="rcp", bufs=2))
        osb_p = ctx.enter_context(tc.tile_pool(name="osb", bufs=2))
        bc_p = ctx.enter_context(tc.tile_pool(name="bcp", bufs=2))
        ysb_p = ctx.enter_context(tc.tile_pool(name="ysb", bufs=2))
        ps_big = ctx.enter_context(tc.tile_pool(name="psbig", bufs=2, space="PSUM"))
        ps_s = ctx.enter_context(tc.tile_pool(name="pss", bufs=4, space="PSUM"))
        ps_o = ctx.enter_context(tc.tile_pool(name="pso", bufs=2, space="PSUM"))

        # ---- resident tiles ----
        xt_all = persist.tile([128, 3, BPC, KT, L], f16, tag="xt_all")
        wv_sb = persist.tile([128, KT, HID], f16, tag="wv_sb")
        wo_sb = persist.tile([128, KT, HID], f16, tag="wo_sb")
        bqk_sb = persist.tile([128, 6, KT], f16, tag="bqk_sb")
        bv_sb = persist.tile([1, HID], f16, tag="bv_sb")
        bo_sb = persist.tile([1, HID], f16, tag="bo_sb")
        ones1 = persist.tile([1, 128], f16, tag="ones1")
        expb = persist.tile([128, 1], f32, tag="expb")
        v_aug = persist.tile([128, BPC, 4, 16 * 80], f16, tag="v_aug")
        ot_all = persist.tile([128, BPC, NJ, L], f16, tag="ot_all")

        nc.vector.memset(ones1[:], 1.0)
        nc.vector.memset(expb[:], -SHIFT)

        # ---- V projection: V[q, n] (natural layout), packed as [q, 16*(64+1)]
        # with a ones column per head for the softmax denominator.  The V input
        # tile lives in its own pool, released after this phase. ----
        with tc.tile_pool(name="xtv", bufs=1) as xtv_pool:
            xt_v = xtv_pool.tile([128, BPC, KT, L], f16, tag="xt_v")
            for b in range(BPC):
                nc.sync.dma_start(out=xt_v[:, b], in_=xt[3, b])
            nc.sync.dma_start(out=wv_sb[:], in_=wv[:])
            for src in range(3):
                for b in range(BPC):
                    nc.sync.dma_start(out=xt_all[:, src, b], in_=xt[src, b])
            nc.sync.dma_start(out=wo_sb[:], in_=wo[:])
            nc.sync.dma_start(out=bqk_sb[:], in_=bqk[:])
            nc.sync.dma_start(out=bv_sb[:], in_=bv[None, :])
            nc.sync.dma_start(out=bo_sb[:], in_=bo[None, :])

            for b in range(BPC):
                v_aug_b = v_aug[:, b].rearrange("p t (h c) -> p t h c", c=80)
                for qt in range(4):
                    nc.vector.memset(v_aug_b[:, qt, :, 64:65], 1.0)
                    for nh in range(2):
                        ps = ps_big.tile([128, 512], f32, tag="psbig", name="psv")
                        for kt in range(KT):
                            nc.tensor.matmul(
                                ps[:],
                                lhsT=xt_v[:, b, kt, qt * 128:(qt + 1) * 128],
                                rhs=wv_sb[:, kt, nh * 512:(nh + 1) * 512],
                                start=(kt == 0), stop=False,
                            )
                        nc.tensor.matmul(
                            ps[:], lhsT=ones1[:], rhs=bv_sb[:, nh * 512:(nh + 1) * 512],
                            start=False, stop=True,
                        )
                        nc.vector.tensor_copy(
                            v_aug_b[:, qt, nh * 8:(nh + 1) * 8, 0:64],
                            ps[:].rearrange("p (h d) -> p h d", d=64),
                        )

        # ---- per head-pair pipeline ----
        def emit_proj(wsl, j, b):
            """Six projections (Qid,Kid,Qc,Kc,Qp,Kp) for head pair j, batch b."""
            qk = []
            for w6 in range(6):
                src = w6 // 2
                ps = ps_big.tile([128, 512], f32, tag="psbig", name="psp")
                for kt in range(KT):
                    nc.tensor.matmul(
                        ps[:],
                        lhsT=wsl[:, w6, kt],
                        rhs=xt_all[:, src, b, kt],
                        start=(kt == 0), stop=(kt == KT - 1),
                    )
                t = qkt_p.tile([128, 512], f16, tag="qkt", name="qkt")
                is_q = (w6 % 2 == 0)
                nc.scalar.activation(
                    t[:], ps[:], AF.Identity,
                    bias=bqk_sb[:, w6, j:j + 1],
                    scale=(float(HD) ** -0.5 if is_q else 1.0),
                )
                qk.append(t)
            return qk

        def emit_scores(qk, pss, kts):
            for si in range(3):
                for h01 in range(2):
                    sl = slice(64 * h01, 64 * (h01 + 1))
                    nc.tensor.matmul(
                        pss[h01][:],
                        lhsT=qk[2 * si + 1][sl, kts * 128:(kts + 1) * 128],
                        rhs=qk[2 * si][sl, :],
                        start=(si == 0), stop=(si == 2),
                        tile_position=(64 * h01, 0),
                    )

        def emit_softmax(pss, rel, kts):
            es = []
            for h01 in range(2):
                nc.vector.tensor_add(pss[h01][:], pss[h01][:], rel[h01][:, kts])
                e = e_p.tile([128, 512], f16, tag="ep", name="e")
                nc.scalar.activation(e[:], pss[h01][:], AF.Exp, bias=expb[:])
                es.append(e)
            return es

        def emit_pv(po, es, v_aug_b, j, b, kts):
            for h01 in range(2):
                nc.tensor.matmul(
                    po[h01][:],
                    lhsT=v_aug_b[:, kts, 2 * j + h01, 0:65],
                    rhs=es[h01][:],
                    start=(kts == 0), stop=(kts == 3),
                )

        def emit_normalize(po, j, b):
            for h01 in range(2):
                # Evacuate [O_unnorm | D] to SBUF right away (frees the PSUM bank
                # for the next pair's PV accumulation), then normalize entirely
                # from SBUF where every step has a full iteration of slack:
                # fast seed+Newton reciprocal on DVE, partition-broadcast via
                # SBUF->SBUF DMA, and the final scale writes O^T in fp16.
                osb = osb_p.tile([65, 512], f32, tag="osb", name="osb")
                nc.scalar.copy(osb[:], po[h01][:])
                # custom-DVE op needs a partition-0 SBUF operand: stage D alone
                dsb = rc_p.tile([1, 512], f32, tag="dsb", name="dsb")
                nc.scalar.copy(dsb[:], po[h01][64:65, :])
                rc = rc_p.tile([1, 512], f32, tag="rcp", name="rc")
                nc.vector.reciprocal_approx_fast(rc[:], dsb[:])
                bc = bc_p.tile([64, 512], f32, tag="bcp", name="bc")
                nc.sync.dma_start(
                    out=bc[:], in_=rc[0:1, None, :].broadcast_to([1, 64, 512])
                )
                nc.vector.tensor_mul(
                    ot_all[64 * h01:64 * (h01 + 1), b, j, :],
                    osb[0:64, :],
                    bc[:],
                )

        def finish_pair(pending):
            """Deferred last PV matmul + normalize for the previous head pair —
            emitted after the next pair's projection matmuls so the PE queue
            never head-of-line blocks on the softmax chain."""
            ppo, pes, pj, pb = pending
            emit_pv(
                ppo, pes,
                v_aug[:, pb].rearrange("p t (h c) -> p t h c", c=80),
                pj, pb, 3,
            )
            emit_normalize(ppo, pj, pb)

        wsl = None
        pending = None
        for j in range(NJ):
            for b in range(BPC):
                if b == 0:
                    wsl = wslices.tile([128, 6, KT, 128], f16, tag="wsl", name="wsl")
                    nc.sync.dma_start(out=wsl[:], in_=wqk[j])
                qk = emit_proj(wsl, j, b)

                if pending is not None:
                    finish_pair(pending)

                rel = []
                for h01 in range(2):
                    rt = rel_p.tile([128, 4, 512], f16, tag="relp", name="rel")
                    nc.sync.dma_start(out=rt[:], in_=relt[b, 2 * j + h01])
                    rel.append(rt)

                v_aug_b = v_aug[:, b].rearrange("p t (h c) -> p t h c", c=80)
                po = [ps_o.tile([65, 512], f32, tag="pso", name="po") for _ in range(2)]
                es_by_kts = []
                for kts in range(4):
                    pss = [ps_s.tile([128, 512], f32, tag="pss", name="pss") for _ in range(2)]
                    emit_scores(qk, pss, kts)
                    es_by_kts.append(emit_softmax(pss, rel, kts))
                    if kts >= 1:
                        emit_pv(po, es_by_kts[kts - 1], v_aug_b, j, b, kts - 1)
                pending = (po, es_by_kts[3], j, b)
        finish_pair(pending)

        # ---- output projection: y[q, n] ----
        for b in range(BPC):
            for qt in range(4):
                for nh in range(2):
                    ps = ps_big.tile([128, 512], f32, tag="psbig", name="psy")
                    for jj in range(NJ):
                        nc.tensor.matmul(
                            ps[:],
                            lhsT=ot_all[:, b, jj, qt * 128:(qt + 1) * 128],
                            rhs=wo_sb[:, jj, nh * 512:(nh + 1) * 512],
                            start=(jj == 0), stop=False,
                        )
                    nc.tensor.matmul(
                        ps[:], lhsT=ones1[:], rhs=bo_sb[:, nh * 512:(nh + 1) * 512],
                        start=False, stop=True,
                    )
                    ysb = ysb_p.tile([128, 512], f32, tag="ysb", name="ysb")
                    nc.scalar.copy(ysb[:], ps[:])
                    nc.sync.dma_start(
                        out=y[b, qt * 128:(qt + 1) * 128, nh * 512:(nh + 1) * 512],
                        in_=ysb[:],
                    )

    nc.finalize()
    return nc


def prep_inputs(inputs):
    """Host-side sharding + layout prep. Returns per-core in_maps.

    Every device tensor is laid out partition-major so DMAs are linear:
    the value at SBUF (partition p, ...) sits contiguously in DRAM.
    """
    f16 = np.float16
    inputs = {k: np.asarray(v) for k, v in inputs.items()}
    s = float(HD) ** -0.5

    # xt: [4, B, 128p, KT, L] where (kt*128+p) indexes HID of x^T [HID, L]
    xt_full = np.empty((4, B, 128, KT, L), f16)
    for i, k in enumerate(("seq_id", "seq_cate", "seq_pos", "V_id_input")):
        x = inputs[k].astype(f16)                       # [B, L, HID]
        xt = x.transpose(0, 2, 1)                       # [B, HID, L]
        xt_full[i] = xt.reshape(B, KT, 128, L).transpose(0, 2, 1, 3)

    # wqk: [NJ, 128p, 6, KT, 128n] — per head-pair column slices of the six
    # Q/K weight matrices, hid_in = kt*128+p.
    wqk_st = np.stack(
        [inputs[k] for k in ("q_id_w", "k_id_w", "q_cate_w", "k_cate_w", "q_pos_w", "k_pos_w")]
    ).astype(f16)                                       # [6, HID, HID]
    wqk_r = wqk_st.reshape(6, KT, 128, NJ, 128)          # [6, kt, p, j, n]
    wqk_lin = np.ascontiguousarray(wqk_r.transpose(3, 2, 0, 1, 4))  # [j, p, 6, kt, n]

    def w_lin(w):  # [HID, HID] -> [128p, KT, HID]
        return np.ascontiguousarray(
            w.astype(f16).reshape(KT, 128, HID).transpose(1, 0, 2)
        )

    wv_lin = w_lin(inputs["v_id_w"])
    wo_lin = w_lin(inputs["out_w"])

    bqk_st = np.stack(
        [
            inputs["q_id_b"] * s, inputs["k_id_b"],
            inputs["q_cate_b"] * s, inputs["k_cate_b"],
            inputs["q_pos_b"] * s, inputs["k_pos_b"],
        ]
    ).astype(f16)                                       # [6, HID]
    bqk_lin = np.ascontiguousarray(
        bqk_st.reshape(6, KT, 128).transpose(2, 0, 1)   # [128p, 6, kt]
    )
    bv_h = inputs["v_id_b"].astype(f16)
    bo_h = inputs["out_b"].astype(f16)

    # relt: [B, NH, 128p, 4kts, L] with (kts*128+p) indexing k of rel^T [k, q]
    relT = np.empty((B, NH, 128, 4, L), f16)
    for b in range(B):
        maskadd = np.where(inputs["attn_mask"][b], np.float32(0), np.float32(MASKVAL))
        relb = inputs["relative_time"][b].astype(np.float32) + maskadd[None]
        rT = relb.transpose(0, 2, 1).astype(f16)         # [NH, k, q]
        relT[b] = rT.reshape(NH, 4, 128, L).transpose(0, 2, 1, 3)

    in_maps = []
    for c in range(NCORES):
        bs = slice(c * BPC, (c + 1) * BPC)
        in_maps.append(
            {
                "xt": np.ascontiguousarray(xt_full[:, bs]),
                "wqk": wqk_lin, "wv": wv_lin, "wo": wo_lin,
                "bqk": bqk_lin, "bv": bv_h, "bo": bo_h,
                "relt": np.ascontiguousarray(relT[bs]),
            }
        )
    return in_maps


def kernel(**inputs):
    from concourse.bass_utils import run_bass_kernel_spmd

    if "nc" not in _CACHE:
        _CACHE["nc"] = build_bass()
    nc = _CACHE["nc"]
    in_maps = prep_inputs(inputs)
    res = run_bass_kernel_spmd(nc, in_maps, list(range(NCORES)))
    out = np.concatenate([res.results[c]["y"] for c in range(NCORES)], axis=0)
    return out.astype(np.float32)
